# revision 5
# baseline (speedup 1.0000x reference)
"""Trainium2 Bass kernel for nn_Encoder (moe_routing).

Reference semantics:
  keys = vocab_ids*B + batch_id; uniq = sorted unique (padded with 0 to N)
  summed[u] = sum of embeddings of tokens with key uniq[u]
  h = tanh(summed @ W1 + b1)
  encoded[u] = sigmoid(h[u] @ encoder_weights[uniq_vocab[u]])
  returns (ids (N,2) int, encoded (N,4) f32)

Strategy (8 NeuronCores):
  Host computes the routing metadata (argsort of keys, unique segments,
  per-segment member token indices, per-segment vocab row) — pure index
  work, the moe "router".  Output segments are sharded contiguously
  across the 8 cores (2048 segments each).  Each core runs one Bass/Tile
  kernel over 16 tiles of 128 segments:
    - K indirect-DMA gathers of member embedding rows (dummy -> zero row)
    - segment sum (vector adds), TensorE transpose, fp32 matmul with W1
    - bias + tanh, indirect gather of encoder rows, per-partition
      multiply+reduce matvec (D_OUT=4), sigmoid, DMA out.
  ids are emitted host-side (index metadata); encoded comes from device.
"""

import os
import sys
import numpy as np

B, S, D_IN, D_MID, D_OUT, V = 32, 512, 768, 128, 4, 100000
N = B * S  # 16384
NCORES = 8
SEG_PER_CORE = N // NCORES  # 2048
P = 128
TILES = SEG_PER_CORE // P  # 16


def _install_patches():
    """Environment shims: NTFF profile hook (missing antenv.axon_hooks in
    this image) and a TileContext tail-drain fix (this walrus build rejects
    >2 sem waits on one CTRL instruction)."""
    import types
    import antenv

    if not hasattr(antenv, "axon_hooks"):
        mod = types.ModuleType("antenv.axon_hooks")
        _hook = [None]
        mod.set_axon_ntff_profile_hook = lambda h: _hook.__setitem__(0, h)
        mod.get_axon_ntff_profile_hook = lambda: _hook[0]
        sys.modules["antenv.axon_hooks"] = mod
        antenv.axon_hooks = mod
        try:
            sys.path.insert(0, "/root/.axon_site/trn_agent_boot")
            from trn_boot import _ntff_profile_via_ctypes

            hook = _ntff_profile_via_ctypes("/opt/axon/libaxon_pjrt.so")
            if hook is not None:
                mod.set_axon_ntff_profile_hook(hook)
        except Exception:
            pass

    import concourse.tile as tile
    import concourse.mybir as mybir
    from concourse.vector_clock import ScopedClock

    if getattr(tile.TileContext._drain_and_barrier, "_split_patch", False):
        return

    def _drain_and_barrier_split(self, tick_clock, wait_clock):
        nc = self.nc
        probe = nc.sync.nop(nofuse=True)
        wait_clock.add_sem_waits(
            probe.ins, ScopedClock({None: tick_clock.global_clock})
        )
        si = probe.ins.sync_info
        waits = list(si.on_wait or []) if si is not None else []
        if len(waits) > 1:
            si.on_wait = waits[:1]
            for w in waits[1:]:
                extra = nc.sync.nop(nofuse=True)
                if extra.ins.sync_info is None:
                    extra.ins.sync_info = mybir.SyncInfo(on_wait=[w], on_update=[])
                else:
                    extra.ins.sync_info.on_wait = [w]
        nc.sync.drain()
        nc.all_engine_barrier()
        assert self.sems is not None
        popped = nc._tile_sem_poison_stack.pop()
        assert popped is self._sem_poison
        nc.clear_and_free_semaphores(list(self.sems.allocated().values()))
        nc.all_engine_barrier()

    _drain_and_barrier_split._split_patch = True
    tile.TileContext._drain_and_barrier = _drain_and_barrier_split


_NC_CACHE = {}
LAST_RESULT = None  # BassKernelResults of the most recent run (for profiling)

_MAXW = 1  # this walrus build allows only 1 sem wait per instruction


def _split_excess_waits(nc):
    """Move excess per-instruction sem waits onto preceding same-engine
    nops (the engine queue executes in program order, so semantics are
    preserved)."""
    import bass_rust
    import concourse.mybir as mybir

    cnt = 0
    for f in nc.m.functions:
        for bb in f.blocks:
            new_insts = []
            changed = False
            for inst in bb.instructions:
                si = inst.sync_info
                waits = list(si.on_wait) if (si is not None and si.on_wait) else []
                if len(waits) > _MAXW:
                    changed = True
                    si.on_wait = waits[-_MAXW:]
                    rest = waits[:-_MAXW]
                    for j in range(0, len(rest), _MAXW):
                        nop = bass_rust.InstNoOp(
                            name=f"I-wsplit-{cnt}", ins=[], outs=[])
                        cnt += 1
                        nop.engine = inst.engine
                        nop.sync_info = mybir.SyncInfo(
                            on_wait=rest[j:j + _MAXW], on_update=[])
                        new_insts.append(nop)
                new_insts.append(inst)
            if changed:
                bb.instructions = new_insts


def _build_nc(kpass):
    """Build the uniform per-core Bass program (kpass gather passes)."""
    import concourse.bass as bass
    import concourse.mybir as mybir
    import concourse.tile as tile
    from concourse.masks import make_identity

    f32 = mybir.dt.float32
    i32 = mybir.dt.int32
    AF = mybir.ActivationFunctionType
    ALU = mybir.AluOpType
    AX = mybir.AxisListType

    nc = bass.Bass("TRN2", target_bir_lowering=False, debug=False,
                   num_devices=NCORES)

    emb = nc.dram_tensor("emb", [N + 1, D_IN], f32, kind="ExternalInput")
    w1 = nc.dram_tensor("w1", [D_IN, D_MID], f32, kind="ExternalInput")
    b1b = nc.dram_tensor("b1b", [P, D_MID], f32, kind="ExternalInput")
    enc = nc.dram_tensor("enc", [V, D_MID * D_OUT], f32, kind="ExternalInput")
    toks = [
        nc.dram_tensor(f"tok{k}", [P, TILES], i32, kind="ExternalInput")
        for k in range(kpass)
    ]
    vidx = nc.dram_tensor("vidx", [P, TILES], i32, kind="ExternalInput")
    out = nc.dram_tensor("out", [SEG_PER_CORE, D_OUT], f32, kind="ExternalOutput")

    with tile.TileContext(nc) as tc:
        with (
            tc.tile_pool(name="const", bufs=1) as const_pool,
            tc.tile_pool(name="g", bufs=3) as g_pool,
            tc.tile_pool(name="work", bufs=2) as work_pool,
            tc.tile_pool(name="psum", bufs=2, space="PSUM") as psum_pool,
            tc.tile_pool(name="psum_y", bufs=2, space="PSUM") as psum_y_pool,
        ):
            ident = const_pool.tile([P, P], f32)
            make_identity(nc, ident[:])
            w1_sb = const_pool.tile([P, 6 * P], f32, tag="w1sb")
            for c in range(6):
                nc.sync.dma_start(
                    out=w1_sb[:, c * P:(c + 1) * P],
                    in_=w1[c * P:(c + 1) * P, :],
                )
            b1_sb = const_pool.tile([P, D_MID], f32, tag="b1sb")
            nc.sync.dma_start(out=b1_sb[:], in_=b1b[:])
            tok_sb = [const_pool.tile([P, TILES], i32, tag=f"tok{k}",
                                      name=f"tok_sb{k}")
                      for k in range(kpass)]
            for k in range(kpass):
                nc.sync.dma_start(out=tok_sb[k][:], in_=toks[k][:])
            vidx_sb = const_pool.tile([P, TILES], i32, tag="vidx")
            nc.sync.dma_start(out=vidx_sb[:], in_=vidx[:])

            for t in range(TILES):
                # --- gather member embedding rows & segment-sum ---
                gs = []
                for k in range(kpass):
                    g = g_pool.tile([P, D_IN], f32, tag=f"g{k}")
                    nc.gpsimd.indirect_dma_start(
                        out=g[:],
                        out_offset=None,
                        in_=emb[:],
                        in_offset=bass.IndirectOffsetOnAxis(
                            ap=tok_sb[k][:, t:t + 1], axis=0
                        ),
                    )
                    gs.append(g)
                e = work_pool.tile([P, D_IN], f32, tag="e")
                if kpass == 1:
                    e = gs[0]
                else:
                    nc.vector.tensor_tensor(
                        out=e[:], in0=gs[0][:], in1=gs[1][:], op=ALU.add
                    )
                    for k in range(2, kpass):
                        nc.vector.tensor_tensor(
                            out=e[:], in0=e[:], in1=gs[k][:], op=ALU.add
                        )

                # --- transpose E chunks, matmul with W1 -> Y (seg x mid) ---
                et = work_pool.tile([P, 6 * P], f32, tag="et")
                for c in range(6):
                    tp = psum_pool.tile([P, P], f32, tag="tp")
                    nc.tensor.transpose(
                        out=tp[:], in_=e[:, c * P:(c + 1) * P], identity=ident[:]
                    )
                    nc.scalar.copy(out=et[:, c * P:(c + 1) * P], in_=tp[:])
                y = psum_y_pool.tile([P, D_MID], f32, tag="y")
                for c in range(6):
                    nc.tensor.matmul(
                        out=y[:],
                        lhsT=et[:, c * P:(c + 1) * P],
                        rhs=w1_sb[:, c * P:(c + 1) * P],
                        start=(c == 0),
                        stop=(c == 5),
                    )

                # --- h = tanh(y + b1) ---
                h = work_pool.tile([P, D_MID], f32, tag="h")
                nc.vector.tensor_tensor(out=h[:], in0=y[:], in1=b1_sb[:], op=ALU.add)
                nc.scalar.activation(h[:], h[:], AF.Tanh)

                # --- gather encoder rows, matvec over D_MID, sigmoid ---
                wt = work_pool.tile([P, D_MID * D_OUT], f32, tag="wt")
                nc.gpsimd.indirect_dma_start(
                    out=wt[:],
                    out_offset=None,
                    in_=enc[:],
                    in_offset=bass.IndirectOffsetOnAxis(
                        ap=vidx_sb[:, t:t + 1], axis=0
                    ),
                )
                wt_v = wt[:].rearrange("p (i o) -> p o i", o=D_OUT)
                acc = work_pool.tile([P, D_OUT], f32, tag="acc")
                prod = work_pool.tile([P, D_MID], f32, tag="prod")
                for o in range(D_OUT):
                    nc.vector.tensor_tensor(
                        out=prod[:], in0=h[:], in1=wt_v[:, o, :], op=ALU.mult
                    )
                    nc.vector.tensor_reduce(
                        out=acc[:, o:o + 1], in_=prod[:], axis=AX.X, op=ALU.add
                    )
                nc.scalar.activation(acc[:], acc[:], AF.Sigmoid)
                nc.sync.dma_start(
                    out=out[t * P:(t + 1) * P, :], in_=acc[:]
                )

    _split_excess_waits(nc)
    return nc


def kernel(vocab_ids, embeddings, W1, b1, encoder_weights):
    _install_patches()
    from concourse.bass_utils import run_bass_kernel_spmd

    global LAST_RESULT

    vocab_ids = np.asarray(vocab_ids)
    embeddings = np.ascontiguousarray(np.asarray(embeddings, dtype=np.float32))
    W1 = np.ascontiguousarray(np.asarray(W1, dtype=np.float32))
    b1 = np.asarray(b1, dtype=np.float32)
    encoder_weights = np.ascontiguousarray(
        np.asarray(encoder_weights, dtype=np.float32))

    # ---------- host routing (index metadata only) ----------
    keys = (vocab_ids.astype(np.int64) * B
            + np.arange(B, dtype=np.int64)[:, None]).reshape(-1)
    order = np.argsort(keys, kind="stable").astype(np.int64)
    sk = keys[order]
    new_seg = np.empty(N, dtype=bool)
    new_seg[0] = True
    new_seg[1:] = sk[1:] != sk[:-1]
    seg_of_pos = np.cumsum(new_seg) - 1          # segment per sorted position
    U = int(seg_of_pos[-1]) + 1
    starts = np.nonzero(new_seg)[0]
    occ = np.arange(N) - starts[seg_of_pos]      # occurrence rank in segment
    kpass = int(occ.max()) + 1

    tok = np.full((kpass, N), N, dtype=np.int32)  # dummy -> zero row N
    tok[occ, seg_of_pos] = order.astype(np.int32)
    uniq_keys = sk[starts]
    uvocab = (uniq_keys // B).astype(np.int64)
    ubatch = (uniq_keys % B).astype(np.int64)

    ids = np.zeros((N, 2), dtype=vocab_ids.dtype)
    ids[:U, 0] = uvocab
    ids[:U, 1] = ubatch

    vfull = np.zeros(N, dtype=np.int32)
    vfull[:U] = uvocab

    emb_pad = np.zeros((N + 1, D_IN), dtype=np.float32)
    emb_pad[:N] = embeddings.reshape(N, D_IN)
    b1b = np.ascontiguousarray(np.broadcast_to(b1, (P, D_MID)))
    enc2d = encoder_weights.reshape(V, D_MID * D_OUT)

    # ---------- shard per core ----------
    def core_layout(arr_n, c):
        # (SEG_PER_CORE,) slice -> (P, TILES) with [p, t] = seg c*SPC + t*P + p
        sl = arr_n[c * SEG_PER_CORE:(c + 1) * SEG_PER_CORE]
        return np.ascontiguousarray(sl.reshape(TILES, P).T)

    in_maps = []
    for c in range(NCORES):
        m = {"emb": emb_pad, "w1": W1, "b1b": b1b, "enc": enc2d,
             "vidx": core_layout(vfull, c)}
        for k in range(kpass):
            m[f"tok{k}"] = core_layout(tok[k], c)
        in_maps.append(m)

    # ---------- build + run ----------
    key = ("v1", kpass)
    if key not in _NC_CACHE:
        _NC_CACHE[key] = _build_nc(kpass)
    nc = _NC_CACHE[key]

    res = run_bass_kernel_spmd(nc, in_maps, list(range(NCORES)))
    LAST_RESULT = res

    encoded = np.concatenate([res.results[c]["out"] for c in range(NCORES)],
                             axis=0)
    return ids, encoded
